# revision 2
# baseline (speedup 1.0000x reference)
"""CBAM (channel + spatial attention) Trainium2 Bass kernel.

Full inputs:  x [32, 512, 56, 56] f32, w1 [512, 32], w2 [32, 512],
              conv_w [1, 2, 7, 7].
Sharding: data-parallel over batch — 4 images per core on 8 cores; params
replicated (small derived weight tensors are precomputed on host).

Per-core layout: each image is held in SBUF as [C=4x128 partitions, HW=3136
free].  Channel stats: per-channel sum rides an ACT Copy pass (accum_out),
per-channel max is a DVE free-dim reduce.  The tiny MLP runs on the PE.
x*att is an ACT Copy with per-partition scale.  Spatial mean over channels is
a ones-weighted PE matmul folded with the 1/512 into the conv band weights;
spatial max is TT-max chunk combine + PE transposes + DVE reduces.  The 7x7
conv over the 2-channel [mean,max] map is 7 accumulated PE matmuls against
host-built banded weight matrices.  The sigmoid spatial map is broadcast
across partitions with K=1 outer-product matmuls and applied with DVE
tensor-tensor multiplies.
"""

import numpy as np
from contextlib import ExitStack

B = 32
C = 512
H = 56
W = 56
HW = H * W  # 3136
CH = C // 16  # 32 hidden
K = 7
PAD = 3
NCORES = 8
PER = B // NCORES  # 4 images per core
NCH = 4  # channel chunks of 128
P = 128
TB = 112  # transpose block width (28 blocks of 112 = 3136)
NTB = HW // TB  # 28
PADW = W + 2 * PAD  # 62

WAIT_LIMIT = 1

_CACHE = {}


def _cap_sync_waits(nc, mybir, limit=WAIT_LIMIT):
    """This walrus build rejects instructions carrying more than `limit` sem
    waits; hoist the excess onto same-engine nops placed just before."""
    cur_list = nc.cur_bb.bb.instructions
    for fn in nc.m.functions:
        for bb in fn.blocks:
            lst = bb.instructions
            i = 0
            while i < len(lst):
                inst = lst[i]
                si = inst.sync_info
                if si is not None and si.on_wait and len(si.on_wait) > limit:
                    waits = list(si.on_wait)
                    keep = waits[-limit:]
                    excess = waits[:-limit]
                    nops = []
                    for j in range(0, len(excess), limit):
                        chunk = excess[j : j + limit]
                        nc.engines[inst.engine].nop()
                        ni = cur_list.pop()
                        ni.sync_info = mybir.SyncInfo(on_wait=chunk, on_update=[])
                        nops.append(ni)
                    inst.sync_info = mybir.SyncInfo(
                        on_wait=keep, on_update=list(si.on_update or [])
                    )
                    lst[i:i] = nops
                    i += len(nops)
                i += 1


def _build_nc():
    import concourse.bass as bass
    import concourse.tile as tile
    from concourse import mybir

    f32 = mybir.dt.float32
    AF = mybir.ActivationFunctionType
    OP = mybir.AluOpType
    AX = mybir.AxisListType

    nc = bass.Bass("TRN2", target_bir_lowering=False, debug=False,
                   enable_asserts=False)

    x_d = nc.dram_tensor("x", [PER, C, HW], f32, kind="ExternalInput").ap()
    w1c_d = nc.dram_tensor("w1cat", [P, 2 * NCH, CH], f32, kind="ExternalInput").ap()
    w2_d = nc.dram_tensor("w2", [CH, C], f32, kind="ExternalInput").ap()
    cb_d = nc.dram_tensor("convband", [2 * PADW, K, H], f32, kind="ExternalInput").ap()
    ones_d = nc.dram_tensor("onescol", [P, 1], f32, kind="ExternalInput").ap()
    onesr_d = nc.dram_tensor("onesrow", [1, P], f32, kind="ExternalInput").ap()
    id_d = nc.dram_tensor("id128", [P, P], f32, kind="ExternalInput").ap()
    y_d = nc.dram_tensor("y", [PER, C, HW], f32, kind="ExternalOutput").ap()

    # [b, (c4 p), hw] -> [b, p, c4, hw] so one DMA per image fills the big tile
    x_r = x_d.rearrange("b (c4 p) hw -> b p c4 hw", p=P)
    y_r = y_d.rearrange("b (c4 p) hw -> b p c4 hw", p=P)

    with tile.TileContext(nc) as tc:
        with ExitStack() as ctx:
            consts = ctx.enter_context(tc.tile_pool(name="consts", bufs=1))
            bigs = ctx.enter_context(tc.tile_pool(name="bigs", bufs=2))
            scrs = ctx.enter_context(tc.tile_pool(name="scrs", bufs=2))
            sbcs = ctx.enter_context(tc.tile_pool(name="sbcs", bufs=2))
            one_off = ctx.enter_context(tc.tile_pool(name="one_off", bufs=1))
            smalls = ctx.enter_context(tc.tile_pool(name="smalls", bufs=2))

            ps_mlp = ctx.enter_context(tc.tile_pool(name="ps_mlp", bufs=1, space="PSUM"))
            ps_mean = ctx.enter_context(tc.tile_pool(name="ps_mean", bufs=2, space="PSUM"))
            ps_tp = ctx.enter_context(tc.tile_pool(name="ps_tp", bufs=2, space="PSUM"))
            ps_misc = ctx.enter_context(tc.tile_pool(name="ps_misc", bufs=1, space="PSUM"))
            ps_bc = ctx.enter_context(tc.tile_pool(name="ps_bc", bufs=2, space="PSUM"))

            # --- constants ---
            w1c = consts.tile([P, 2 * NCH, CH], f32)
            nc.sync.dma_start(w1c[:], w1c_d)
            w2 = consts.tile([CH, C], f32)
            nc.sync.dma_start(w2[:], w2_d)
            convb = consts.tile([2 * PADW, K, H], f32)
            nc.sync.dma_start(convb[:], cb_d)
            ones = consts.tile([P, 1], f32)
            nc.sync.dma_start(ones[:], ones_d)
            onesr = consts.tile([1, P], f32)
            nc.sync.dma_start(onesr[:], onesr_d)
            iden = consts.tile([P, P], f32)
            nc.sync.dma_start(iden[:], id_d)
            # persistent padded [mean;max] map, rows on partitions:
            # partition ci*62 + y', free x'  (borders stay zero)
            padded = consts.tile([2 * PADW, PADW], f32)
            nc.vector.memset(padded[:], 0.0)
            trash = one_off.tile([P, HW], f32)
            mean_sb = one_off.tile([1, HW], f32)

            for b in range(PER):
                big = bigs.tile([P, NCH, HW], f32, tag="big")
                nc.sync.dma_start(big[:], x_r[b])

                # --- channel stats: sum (ACT accum) + max (DVE reduce) ---
                stats = smalls.tile([P, 2 * NCH], f32, tag="stats")
                for c4 in range(NCH):
                    nc.scalar.activation(
                        trash[:], big[:, c4, :], AF.Copy,
                        accum_out=stats[:, c4 : c4 + 1],
                    )
                    nc.vector.reduce_max(
                        out=stats[:, NCH + c4 : NCH + c4 + 1],
                        in_=big[:, c4, :], axis=AX.X,
                    )

                # --- MLP: att = sigmoid(w2.T @ (relu(w1s.T@sum) + relu(w1.T@max))) ---
                h_ps = ps_mlp.tile([CH, 2], f32, tag="mlp")
                for c4 in range(NCH):
                    nc.tensor.matmul(
                        h_ps[:, 0:1], lhsT=w1c[:, 2 * c4 + 0, :],
                        rhs=stats[:, c4 : c4 + 1],
                        start=(c4 == 0), stop=(c4 == NCH - 1),
                    )
                for c4 in range(NCH):
                    nc.tensor.matmul(
                        h_ps[:, 1:2], lhsT=w1c[:, 2 * c4 + 1, :],
                        rhs=stats[:, NCH + c4 : NCH + c4 + 1],
                        start=(c4 == 0), stop=(c4 == NCH - 1),
                    )
                h_sb = smalls.tile([CH, 2], f32, tag="h_sb")
                nc.scalar.activation(h_sb[:], h_ps[:], AF.Relu)
                hs = smalls.tile([CH, 1], f32, tag="hs")
                nc.vector.tensor_add(hs[:], h_sb[:, 0:1], h_sb[:, 1:2])
                att_ps = ps_mlp.tile([P, NCH], f32, tag="mlp")
                for c4 in range(NCH):
                    nc.tensor.matmul(
                        att_ps[:, c4 : c4 + 1],
                        lhsT=w2[:, c4 * P : (c4 + 1) * P], rhs=hs[:],
                        start=True, stop=True,
                    )
                att_sb = smalls.tile([P, NCH], f32, tag="att_sb")
                nc.scalar.activation(att_sb[:], att_ps[:], AF.Sigmoid)

                # --- out1 = x * att  (in place, ACT per-partition scale) ---
                for c4 in range(NCH):
                    nc.scalar.activation(
                        big[:, c4, :], big[:, c4, :], AF.Copy,
                        scale=att_sb[:, c4 : c4 + 1],
                    )

                # --- spatial mean: ones-matmul over channels (raw sum; the
                # 1/512 is folded into the conv band weights) ---
                NSL = 7
                SL = HW // NSL  # 448
                for k in range(NSL):
                    mean_ps = ps_mean.tile([1, SL], f32, tag="mean")
                    for c4 in range(NCH):
                        nc.tensor.matmul(
                            mean_ps[:],
                            lhsT=ones[:], rhs=big[:, c4, k * SL : (k + 1) * SL],
                            start=(c4 == 0), stop=(c4 == NCH - 1),
                        )
                    nc.scalar.copy(mean_sb[:, k * SL : (k + 1) * SL], mean_ps[:])
                nc.sync.dma_start(padded[PAD : PAD + H, PAD : PAD + W], mean_sb[:])

                # --- spatial max over all 512 channels ---
                scrA = scrs.tile([P, HW], f32, tag="scr")
                scrB = scrs.tile([P, HW], f32, tag="scr")
                nc.vector.tensor_tensor(scrA[:], big[:, 0, :], big[:, 1, :], op=OP.max)
                nc.vector.tensor_tensor(scrB[:], big[:, 2, :], big[:, 3, :], op=OP.max)
                nc.vector.tensor_tensor(scrA[:], scrA[:], scrB[:], op=OP.max)
                r_tile = smalls.tile([TB, NTB], f32, tag="r_tile")
                for blk in range(NTB):
                    tp_ps = ps_tp.tile([TB, P], f32, tag="tp")
                    nc.tensor.transpose(
                        tp_ps[:], scrA[:, blk * TB : (blk + 1) * TB], iden[:]
                    )
                    nc.vector.reduce_max(
                        out=r_tile[:, blk : blk + 1], in_=tp_ps[:], axis=AX.X
                    )
                rq_ps = ps_misc.tile([NTB, TB], f32, tag="misc")
                nc.tensor.transpose(rq_ps[:], r_tile[:], iden[0:TB, 0:TB])
                rq_sb = smalls.tile([NTB, TB], f32, tag="rq_sb")
                nc.scalar.copy(rq_sb[:], rq_ps[:])
                # row blk covers hw [112*blk, 112*blk+112) = image rows 2blk, 2blk+1
                nc.sync.dma_start(
                    padded[PADW + PAD : PADW + PAD + H : 2, PAD : PAD + W],
                    rq_sb[:, 0:W],
                )
                nc.sync.dma_start(
                    padded[PADW + PAD + 1 : PADW + PAD + H + 1 : 2, PAD : PAD + W],
                    rq_sb[:, W : 2 * W],
                )

                # --- 7x7 conv as 7 banded matmuls -> conv_ps[y, x] ---
                conv_ps = ps_misc.tile([H, W], f32, tag="misc")
                for kx in range(K):
                    nc.tensor.matmul(
                        conv_ps[:],
                        lhsT=convb[:, kx, :], rhs=padded[:, kx : kx + W],
                        start=(kx == 0), stop=(kx == K - 1),
                    )
                s_yx = smalls.tile([H, W], f32, tag="s_yx")
                nc.scalar.copy(s_yx[:], conv_ps[:])
                s_row = smalls.tile([1, HW], f32, tag="s_row")
                nc.sync.dma_start(s_row[:], s_yx[:])

                # --- broadcast across partitions (K=1 matmul), sigmoid -> s_bcast ---
                s_bcast = sbcs.tile([P, HW], f32, tag="sbc")
                for k in range(NSL):
                    bc_ps = ps_bc.tile([P, SL], f32, tag="bc")
                    nc.tensor.matmul(
                        bc_ps[:],
                        lhsT=onesr[:], rhs=s_row[:, k * SL : (k + 1) * SL],
                        start=True, stop=True,
                    )
                    nc.scalar.activation(
                        s_bcast[:, k * SL : (k + 1) * SL], bc_ps[:], AF.Sigmoid
                    )

                # --- final: out = out1 * s  (in place) ---
                for c4 in range(NCH):
                    nc.vector.tensor_tensor(
                        big[:, c4, :], big[:, c4, :], s_bcast[:], op=OP.mult
                    )

                nc.sync.dma_start(y_r[b], big[:])

    _cap_sync_waits(nc, mybir)
    return nc


def _host_weights(w1, w2, conv_w):
    w1 = np.asarray(w1, dtype=np.float32)
    w2 = np.asarray(w2, dtype=np.float32)
    conv_w = np.asarray(conv_w, dtype=np.float32)

    # w1cat[p, 2*c4+0, :] = w1[c4*128+p, :] / 3136  (avg path)
    # w1cat[p, 2*c4+1, :] = w1[c4*128+p, :]         (max path)
    w1cat = np.empty((P, 2 * NCH, CH), dtype=np.float32)
    for c4 in range(NCH):
        w1cat[:, 2 * c4 + 0, :] = w1[c4 * P : (c4 + 1) * P, :] / float(HW)
        w1cat[:, 2 * c4 + 1, :] = w1[c4 * P : (c4 + 1) * P, :]

    # banded conv weights: convband[ci*62+yp, kx, y] = w[ci, yp-y, kx]
    # (ci=0 rows carry the 1/512 for the channel mean)
    convband = np.zeros((2 * PADW, K, H), dtype=np.float32)
    for ci in range(2):
        scale = (1.0 / C) if ci == 0 else 1.0
        for yp in range(PADW):
            for y in range(H):
                ky = yp - y
                if 0 <= ky < K:
                    convband[ci * PADW + yp, :, y] = conv_w[0, ci, ky, :] * scale

    return {
        "w1cat": w1cat,
        "w2": np.ascontiguousarray(w2),
        "convband": convband,
        "onescol": np.ones((P, 1), dtype=np.float32),
        "onesrow": np.ones((1, P), dtype=np.float32),
        "id128": np.eye(P, dtype=np.float32),
    }


def kernel(x, w1, w2, conv_w):
    from concourse.bass_utils import run_bass_kernel_spmd

    if "nc" not in _CACHE:
        _CACHE["nc"] = _build_nc()
    nc = _CACHE["nc"]

    x = np.asarray(x, dtype=np.float32)
    shared = _host_weights(w1, w2, conv_w)

    in_maps = []
    for c in range(NCORES):
        shard = np.ascontiguousarray(
            x[c * PER : (c + 1) * PER].reshape(PER, C, HW)
        )
        in_maps.append({"x": shard, **shared})

    res = run_bass_kernel_spmd(nc, in_maps, core_ids=list(range(NCORES)))
    out = np.concatenate(
        [res.results[c]["y"].reshape(PER, C, H, W) for c in range(NCORES)], axis=0
    )
    return out.astype(np.float32)
